# revision 4
# baseline (speedup 1.0000x reference)
"""Self-contained Trainium2 Bass kernel: causal multi-head attention.

Problem: B=2, S=2048, D=1024, H=16 (DK=64), f32, returns (output, attn).

Sharding over 8 NeuronCores: core c handles batch b = c//4 and the 4 heads
4*(c%4) .. 4*(c%4)+4  (data parallel on B, tensor parallel on heads).
Each core computes its heads' QKV projections, causal attention (writing its
slice of the attention-probability tensor), and a partial output projection.
The host sums the 4 partial output projections per batch (TP unshard).

Everything on-device lives in "transposed space": activations are [channel, seq]
so scores are computed directly as s^T[k, q] tiles and the P@V matmul consumes
them with no on-chip transposes. The host hands in pre-transposed inputs and
transposes the outputs back while gathering.

Softmax: scores are bounded (no max subtraction needed; masked entries get a
-1e9 bias so exp underflows to exactly 0).  The denominator is obtained for
free by appending a ones-column to V in the ctx matmul; normalization is a
reciprocal + gpsimd partition_broadcast + one vector multiply which also
produces the f32 attention tile that is DMA'd out.  Only lower-triangle [k,q]
blocks are computed; the upper region relies on the runtime pre-zeroing
output buffers.  If the mask is not exactly causal, a general (slower)
variant computes every block with the full mask.
"""

import sys
import types

if "/opt/trn_rl_repo" not in sys.path:
    sys.path.insert(0, "/opt/trn_rl_repo")

import numpy as np


def _install_ntff_hook():
    """Recreate antenv.axon_hooks (missing in this image) so that
    run_bass_kernel_spmd(trace=True) can capture NTFF profiles."""
    if "antenv.axon_hooks" in sys.modules:
        return
    try:
        from trn_agent_boot.trn_boot import _ntff_profile_via_ctypes
    except ImportError:
        return
    try:
        hook = _ntff_profile_via_ctypes("/opt/axon/libaxon_pjrt.so")
    except OSError:
        hook = None
    mod = types.ModuleType("antenv.axon_hooks")
    mod.get_axon_ntff_profile_hook = lambda: hook
    mod.set_axon_ntff_profile_hook = lambda h: None
    sys.modules["antenv.axon_hooks"] = mod


_install_ntff_hook()

import concourse.bacc as bacc
import concourse.mybir as mybir
import concourse.tile as tile
from concourse import bass_utils

B, S, D, H = 2, 2048, 1024, 16
DK = D // H          # 64
NCORES = 8
HP_CORE = 4          # heads per core
E = HP_CORE * DK     # 256 proj channels per core
NEG = -1e9

F32 = mybir.dt.float32
F32R = mybir.dt.float32r
Exp = mybir.ActivationFunctionType.Exp

_cache = {}


def _build(causal: bool):
    nc = bacc.Bacc("TRN2", target_bir_lowering=False, debug=False,
                   num_devices=NCORES)

    # ---- I/O ----
    xq = nc.dram_tensor("xq", [D, S], F32, kind="ExternalInput")   # q[b].T
    xk = nc.dram_tensor("xk", [D, S], F32, kind="ExternalInput")
    xv = nc.dram_tensor("xv", [D, S], F32, kind="ExternalInput")
    wq = nc.dram_tensor("wq", [D, E], F32, kind="ExternalInput")   # Wq[hsl,:].T
    wk = nc.dram_tensor("wk", [D, E], F32, kind="ExternalInput")
    wv = nc.dram_tensor("wv", [D, E], F32, kind="ExternalInput")
    wo = nc.dram_tensor("wo", [E, D], F32, kind="ExternalInput")   # Wo[:,hsl].T
    bq = nc.dram_tensor("bq", [1, E], F32, kind="ExternalInput")
    bk = nc.dram_tensor("bk", [1, E], F32, kind="ExternalInput")
    bv = nc.dram_tensor("bv", [1, E], F32, kind="ExternalInput")
    bo = nc.dram_tensor("bo", [1, D], F32, kind="ExternalInput")
    onesr = nc.dram_tensor("onesr", [1, 512], F32, kind="ExternalInput")
    onescol = nc.dram_tensor("onescol", [128, HP_CORE], F32, kind="ExternalInput")
    NKT = S // 128  # 16
    if causal:
        maskd = nc.dram_tensor("maskd", [NKT, 128, 128], F32, kind="ExternalInput")
    else:
        maskf = nc.dram_tensor("maskf", [NKT, 128, S], F32, kind="ExternalInput")

    attnT = nc.dram_tensor("attnT", [HP_CORE, S, S], F32, kind="ExternalOutput")
    outT = nc.dram_tensor("outT", [D, S], F32, kind="ExternalOutput")

    NSB = S // 512   # 4 sequence blocks
    ND = D // 128    # 8 contraction slices

    with tile.TileContext(nc) as tc:
        with tc.tile_pool(name="const", bufs=1) as constp:
            # weights: [128(d in slice), ND*E] layout; lhsT/rhs slices come
            # from column ranges
            wo_t = [constp.tile([128, D], F32R, tag=f"wo{i}", name=f"wo{i}") for i in range(2)]
            for i in range(2):
                nc.sync.dma_start(wo_t[i][:], wo[i * 128:(i + 1) * 128, :].bitcast(F32R))
            bo_t = constp.tile([1, D], F32R, tag="bo", name="bo")
            nc.sync.dma_start(bo_t[:], bo[:].bitcast(F32R))
            ones_t = constp.tile([1, 512], F32R, tag="ones", name="ones")
            nc.sync.dma_start(ones_t[:], onesr[:].bitcast(F32R))
            if causal:
                maskd_t = constp.tile([128, NKT * 128], F32, tag="maskd", name="maskd")
                for kt in range(NKT):
                    nc.sync.dma_start(maskd_t[:, kt * 128:(kt + 1) * 128],
                                      maskd[kt, :, :])

            # persistent activation tiles
            qhT = [constp.tile([128, S], F32R, tag=f"qhT{i}", name=f"qhT{i}") for i in range(2)]
            khT = [constp.tile([128, S], F32R, tag=f"khT{i}", name=f"khT{i}") for i in range(2)]
            vha = [constp.tile([128, HP_CORE * (DK + 1)], F32R, tag=f"vha{k}", name=f"vha{k}")
                   for k in range(NKT)]
            ctx = [constp.tile([128, S], F32R, tag=f"ctx{i}", name=f"ctx{i}") for i in range(2)]

            # ---------------- phase 1: projections ----------------
            with (
                tc.tile_pool(name="wproj", bufs=1) as wprojp,
                tc.tile_pool(name="xs", bufs=12) as xsp,
                tc.tile_pool(name="pp", bufs=4, space="PSUM") as pp,
            ):
                wq_t = wprojp.tile([128, ND * E], F32R, tag="wq", name="wq")
                wk_t = wprojp.tile([128, ND * E], F32R, tag="wk", name="wk")
                wv_t = wprojp.tile([128, ND * E], F32R, tag="wv", name="wv")
                for ds in range(ND):
                    nc.sync.dma_start(wq_t[:, ds * E:(ds + 1) * E],
                                      wq[ds * 128:(ds + 1) * 128, :].bitcast(F32R))
                    nc.sync.dma_start(wk_t[:, ds * E:(ds + 1) * E],
                                      wk[ds * 128:(ds + 1) * 128, :].bitcast(F32R))
                    nc.sync.dma_start(wv_t[:, ds * E:(ds + 1) * E],
                                      wv[ds * 128:(ds + 1) * 128, :].bitcast(F32R))
                bq_t = wprojp.tile([1, E], F32R, tag="bq", name="bq")
                bk_t = wprojp.tile([1, E], F32R, tag="bk", name="bk")
                bv_t = wprojp.tile([1, E], F32R, tag="bv", name="bv")
                nc.sync.dma_start(bq_t[:], bq[:].bitcast(F32R))
                nc.sync.dma_start(bk_t[:], bk[:].bitcast(F32R))
                nc.sync.dma_start(bv_t[:], bv[:].bitcast(F32R))
                # qh^T and kh^T: out[e_tile(128) , s] ; lhsT = w[d,e] slice
                for src, wt, bt, dst in ((xq, wq_t, bq_t, qhT),
                                         (xk, wk_t, bk_t, khT)):
                    for sb in range(NSB):
                        xt = []
                        for ds in range(ND):
                            t = xsp.tile([128, 512], F32R, tag="xs", name="xs")
                            nc.sync.dma_start(
                                t[:], src[ds * 128:(ds + 1) * 128,
                                          sb * 512:(sb + 1) * 512].bitcast(F32R))
                            xt.append(t)
                        for hp in range(2):
                            ps = pp.tile([128, 512], F32, tag="pp", name="pp")
                            nc.tensor.matmul(ps[:], bt[0:1, hp * 128:(hp + 1) * 128],
                                             ones_t[:], start=True, stop=False)
                            for ds in range(ND):
                                nc.tensor.matmul(
                                    ps[:],
                                    wt[:, ds * E + hp * 128: ds * E + (hp + 1) * 128],
                                    xt[ds][:],
                                    start=False, stop=(ds == ND - 1))
                            nc.scalar.copy(dst[hp][:, sb * 512:(sb + 1) * 512], ps[:])
                # vh: out[s_tile(128), e] ; lhsT = x^T[d, s_tile], rhs = wv[d, e]
                for sb in range(NSB):
                    xt = []
                    for ds in range(ND):
                        t = xsp.tile([128, 512], F32R, tag="xs", name="xs")
                        nc.sync.dma_start(
                            t[:], xv[ds * 128:(ds + 1) * 128,
                                     sb * 512:(sb + 1) * 512].bitcast(F32R))
                        xt.append(t)
                    for st in range(4):
                        kt = sb * 4 + st
                        ps = pp.tile([128, E], F32, tag="pp", name="pp")
                        nc.tensor.matmul(ps[:], ones_t[0:1, 0:128], bv_t[:],
                                         start=True, stop=False)
                        for ds in range(ND):
                            nc.tensor.matmul(
                                ps[:], xt[ds][:, st * 128:(st + 1) * 128],
                                wv_t[:, ds * E:(ds + 1) * E],
                                start=False, stop=(ds == ND - 1))
                        # scatter 4 heads' 64 cols into [.., 65h .. 65h+64]
                        dstv = vha[kt][:].rearrange("p (h c) -> p h c", c=DK + 1)
                        nc.scalar.copy(
                            dstv[:, :, 0:DK],
                            ps[:].rearrange("p (h c) -> p h c", c=DK))
                        nc.sync.dma_start(
                            dstv[:, :, DK:DK + 1],
                            onescol[:].rearrange("p (h o) -> p h o", o=1).bitcast(F32R))

            # ---------------- phase 2: attention ----------------
            with (
                tc.tile_pool(name="pt", bufs=12) as ptp,
                tc.tile_pool(name="ptd", bufs=8) as ptdp,
                tc.tile_pool(name="stage", bufs=3) as stagep,
                tc.tile_pool(name="rbp", bufs=2) as rbp,
                tc.tile_pool(name="rrow", bufs=2) as rrowp,
                tc.tile_pool(name="pspair", bufs=2, space="PSUM") as pspair,
                tc.tile_pool(name="psdiag", bufs=2, space="PSUM") as psdiag,
                tc.tile_pool(name="psctx", bufs=2, space="PSUM") as psctx,
                tc.tile_pool(name="msk", bufs=4) as mskp,
            ):
                for hp in range(2):
                    for qb in range(NSB):
                        if causal:
                            nfull = 4 * qb
                            diag_kts = [4 * qb + i for i in range(4)]
                        else:
                            nfull = NKT
                            diag_kts = []
                        last_kt = diag_kts[-1] if causal else nfull - 1
                        pairs = [(k, k + 1) for k in range(0, nfull, 2)]
                        psc = [psctx.tile([65, 512], F32, tag="psctx", name="psctx")
                               for _ in range(2)]
                        head_pts = {0: [], 1: []}  # (pt_tile, [(seg, kt)], width)

                        for kt0, kt1 in pairs:
                            for h01 in range(2):
                                lh = 2 * hp + h01
                                pss = pspair.tile([128, 1024], F32, tag="pspair", name="pspair")
                                for seg, kt in ((0, kt0), (1, kt1)):
                                    nc.tensor.matmul(
                                        pss[:, seg * 512:(seg + 1) * 512],
                                        khT[hp][64 * h01:64 * h01 + 64,
                                                kt * 128:(kt + 1) * 128],
                                        qhT[hp][64 * h01:64 * h01 + 64,
                                                qb * 512:(qb + 1) * 512],
                                        start=True, stop=True,
                                        tile_position=(64 * h01, 0))
                                if not causal:
                                    for seg, kt in ((0, kt0), (1, kt1)):
                                        mt = mskp.tile([128, 512], F32, tag="msk", name="msk")
                                        nc.sync.dma_start(
                                            mt[:], maskf[kt, :, qb * 512:(qb + 1) * 512])
                                        nc.vector.tensor_add(
                                            pss[:, seg * 512:(seg + 1) * 512],
                                            pss[:, seg * 512:(seg + 1) * 512], mt[:])
                                ptt = ptp.tile([128, 1024], F32R, tag="pt", name="pt")
                                nc.scalar.activation(ptt[:], pss[:], Exp, scale=0.125)
                                for seg, kt in ((0, kt0), (1, kt1)):
                                    nc.tensor.matmul(
                                        psc[h01][0:65, :],
                                        vha[kt][:, 65 * lh:65 * lh + 65],
                                        ptt[:, seg * 512:(seg + 1) * 512],
                                        start=(kt == 0), stop=(kt == last_kt))
                                head_pts[h01].append((ptt, [(0, kt0), (1, kt1)], 1024, 0))

                        for kt in diag_kts:
                            coloff = 128 * kt - 512 * qb
                            n = 512 - coloff
                            for h01 in range(2):
                                lh = 2 * hp + h01
                                psd = psdiag.tile([128, n], F32, tag="psdiag", name="psdiag")
                                nc.tensor.matmul(
                                    psd[:, 0:n],
                                    khT[hp][64 * h01:64 * h01 + 64,
                                            kt * 128:(kt + 1) * 128],
                                    qhT[hp][64 * h01:64 * h01 + 64,
                                            qb * 512 + coloff:(qb + 1) * 512],
                                    start=True, stop=True,
                                    tile_position=(64 * h01, 0))
                                nc.vector.tensor_add(
                                    psd[:, 0:128], psd[:, 0:128],
                                    maskd_t[:, kt * 128:(kt + 1) * 128])
                                ptt = ptdp.tile([128, n], F32R, tag="ptd", name="ptd")
                                nc.scalar.activation(ptt[:, 0:n], psd[:, 0:n],
                                                     Exp, scale=0.125)
                                nc.tensor.matmul(
                                    psc[h01][0:65, coloff:512],
                                    vha[kt][:, 65 * lh:65 * lh + 65],
                                    ptt[:, 0:n],
                                    start=(kt == 0), stop=(kt == last_kt))
                                head_pts[h01].append((ptt, [(0, kt)], n, coloff))

                        for h01 in range(2):
                            lh = 2 * hp + h01
                            rrow = rrowp.tile([1, 512], F32, tag="rrow", name="rrow")
                            nc.vector.reciprocal(rrow[0:1, :], psc[h01][64:65, :])
                            rbt = rbp.tile([128, 1024], F32, tag="rb", name="rb")
                            nc.gpsimd.partition_broadcast(rbt[:, 0:512], rrow[0:1, :])
                            nc.gpsimd.partition_broadcast(rbt[:, 512:1024], rrow[0:1, :])
                            # normalized ctx -> persistent ctx tiles (f32r)
                            nc.vector.tensor_mul(
                                ctx[hp][64 * h01:64 * h01 + 64, qb * 512:(qb + 1) * 512],
                                psc[h01][0:64, :], rbt[0:64, 0:512])
                            for ptt, segs, width, coloff in head_pts[h01]:
                                st = stagep.tile([128, width], F32, tag="stage", name="stage")
                                nc.vector.tensor_mul(
                                    st[:, 0:width], ptt[:, 0:width].bitcast(F32),
                                    rbt[:, coloff:coloff + width])
                                for seg, kt in segs:
                                    nc.sync.dma_start(
                                        attnT[lh, kt * 128:(kt + 1) * 128,
                                              qb * 512 + coloff:(qb + 1) * 512],
                                        st[:, seg * 512:seg * 512 + (width if len(segs) == 1 else 512)])

            # ---------------- phase 3: output projection ----------------
            with (
                tc.tile_pool(name="po", bufs=3, space="PSUM") as pop,
                tc.tile_pool(name="ostage", bufs=4) as ostagep,
            ):
                for m in range(D // 128):
                    for qc in range(NSB):
                        ps = pop.tile([128, 512], F32, tag="po", name="po")
                        nc.tensor.matmul(ps[:], bo_t[0:1, m * 128:(m + 1) * 128],
                                         ones_t[:], start=True, stop=False)
                        for i in range(2):
                            nc.tensor.matmul(
                                ps[:], wo_t[i][:, m * 128:(m + 1) * 128],
                                ctx[i][:, qc * 512:(qc + 1) * 512],
                                start=False, stop=(i == 1))
                        ost = ostagep.tile([128, 512], F32, tag="ostage", name="ostage")
                        nc.scalar.copy(ost[:], ps[:])
                        nc.sync.dma_start(outT[m * 128:(m + 1) * 128,
                                               qc * 512:(qc + 1) * 512], ost[:])

    nc.compile()
    return nc


def _get_nc(causal: bool):
    if causal not in _cache:
        _cache[causal] = _build(causal)
    return _cache[causal]


def _prep_inputs(q, k, v, attn_mask, Wq, bq, Wk, bk, Wv, bv, Wo, bo):
    m2 = np.asarray(attn_mask).reshape(S, S)
    causal = bool(np.array_equal((m2 != 0), np.tril(np.ones((S, S), bool))))

    NKT = S // 128
    if causal:
        maskd = np.empty((NKT, 128, 128), np.float32)
        for kt in range(NKT):
            sub = m2[kt * 128:(kt + 1) * 128, kt * 128:(kt + 1) * 128]  # [q,k]
            maskd[kt] = np.where(sub == 0, np.float32(NEG), np.float32(0.0)).T
        mask_inputs = {"maskd": maskd}
    else:
        maskf = np.empty((NKT, 128, S), np.float32)
        for kt in range(NKT):
            sub = m2[:, kt * 128:(kt + 1) * 128]  # [q, k]
            maskf[kt] = np.where(sub == 0, np.float32(NEG), np.float32(0.0)).T
        mask_inputs = {"maskf": maskf}

    xT = {}
    for name, x in (("xq", q), ("xk", k), ("xv", v)):
        xT[name] = [np.ascontiguousarray(np.asarray(x)[b].T) for b in range(B)]

    onesr = np.ones((1, 512), np.float32)
    onescol = np.ones((128, HP_CORE), np.float32)

    Wq, Wk, Wv, Wo = (np.asarray(a, np.float32) for a in (Wq, Wk, Wv, Wo))
    bqv, bkv, bvv, bov = (np.asarray(a, np.float32) for a in (bq, bk, bv, bo))

    in_maps = []
    for c in range(NCORES):
        b = c // 4
        hs = slice(E * (c % 4), E * (c % 4 + 1))
        im = {
            "xq": xT["xq"][b], "xk": xT["xk"][b], "xv": xT["xv"][b],
            "wq": np.ascontiguousarray(Wq[hs, :].T),
            "wk": np.ascontiguousarray(Wk[hs, :].T),
            "wv": np.ascontiguousarray(Wv[hs, :].T),
            "wo": np.ascontiguousarray(Wo[:, hs].T),
            "bq": np.ascontiguousarray(bqv[hs][None, :]),
            "bk": np.ascontiguousarray(bkv[hs][None, :]),
            "bv": np.ascontiguousarray(bvv[hs][None, :]),
            "bo": (bov[None, :].copy() if c % 4 == 0
                   else np.zeros((1, D), np.float32)),
            "onesr": onesr, "onescol": onescol,
        }
        im.update(mask_inputs)
        in_maps.append(im)
    return causal, in_maps


def _gather(results):
    attn = np.empty((B, H, S, S), np.float32)
    output = np.empty((B, S, D), np.float32)
    for b in range(B):
        acc = None
        for g in range(4):
            c = b * 4 + g
            r = results[c]
            at = r["attnT"]
            for i in range(HP_CORE):
                attn[b, HP_CORE * (c % 4) + i] = at[i].T
            acc = r["outT"] if acc is None else acc + r["outT"]
        output[b] = acc.T
    return output, attn


def run(trace=False, **inputs):
    causal, in_maps = _prep_inputs(**inputs)
    nc = _get_nc(causal)
    res = bass_utils.run_bass_kernel_spmd(
        nc, in_maps, core_ids=list(range(NCORES)), trace=trace)
    output, attn = _gather(res.results)
    return (output, attn), res


def kernel(**inputs):
    (output, attn), _ = run(trace=False, **inputs)
    return output, attn


# revision 5
# speedup vs baseline: 1.1167x; 1.1167x over previous
"""Self-contained Trainium2 Bass kernel: causal multi-head attention.

Problem: B=2, S=2048, D=1024, H=16 (DK=64), f32, returns (output, attn).

Sharding over 8 NeuronCores: core c handles batch b = c//4 and the 4 heads
4*(c%4) .. 4*(c%4)+4 (data parallel on B, tensor parallel on heads).  Each
core computes its heads' QKV projections, causal attention (writing its slice
of the attention-probability tensor), and a partial output projection; the
host sums the 4 partial output projections per batch (TP unshard).

On-device layout is "transposed space": activations are [channel, seq], so
score tiles come out as s^T[k, q] and feed the P@V matmul with no on-chip
transposes; the host pre-transposes inputs and de-transposes outputs.

Compute dtype is fp16 (operands; all accumulation f32 in PSUM) — full PE rate
with hideable weight loads.  The causal mask is applied on the PE itself via
an identity-matmul accumulation (psum += I.T @ maskT, mask bias -60000 so exp
underflows to exactly 0).  Softmax denominators come free as a ones-column
appended to V; normalization is reciprocal + gpsimd partition_broadcast + one
multiply that also produces the f32 attention tile.  Only lower-triangle
[k,q] block-rows are computed; the rest relies on pre-zeroed output buffers.
A non-causal mask falls back to a general variant computing every block.
"""

import sys
import types

if "/opt/trn_rl_repo" not in sys.path:
    sys.path.insert(0, "/opt/trn_rl_repo")

import numpy as np


def _install_ntff_hook():
    """Recreate antenv.axon_hooks (missing in this image) so that
    run_bass_kernel_spmd(trace=True) can capture NTFF profiles."""
    if "antenv.axon_hooks" in sys.modules:
        return
    try:
        from trn_agent_boot.trn_boot import _ntff_profile_via_ctypes
    except ImportError:
        return
    try:
        hook = _ntff_profile_via_ctypes("/opt/axon/libaxon_pjrt.so")
    except OSError:
        hook = None
    mod = types.ModuleType("antenv.axon_hooks")
    mod.get_axon_ntff_profile_hook = lambda: hook
    mod.set_axon_ntff_profile_hook = lambda h: None
    sys.modules["antenv.axon_hooks"] = mod


_install_ntff_hook()

import concourse.bacc as bacc
import concourse.mybir as mybir
import concourse.tile as tile
from concourse import bass_utils

B, S, D, H = 2, 2048, 1024, 16
DK = D // H          # 64
NCORES = 8
HPC = 4              # heads per core
E = HPC * DK         # 256 proj channels per core
NEG = -60000.0       # fp16-representable; exp(0.125*(s+NEG)) == 0 in f32
NKT = S // 128       # 16 key tiles
NSB = S // 512       # 4 seq blocks
ND = D // 128        # 8 contraction slices

F32 = mybir.dt.float32
F16 = mybir.dt.float16
Exp = mybir.ActivationFunctionType.Exp

_cache = {}


def _build(causal: bool):
    nc = bacc.Bacc("TRN2", target_bir_lowering=False, debug=False,
                   num_devices=NCORES)

    # ---- I/O (inputs fp16; outputs f32) ----
    xq = nc.dram_tensor("xq", [D, S], F16, kind="ExternalInput")   # q[b].T
    xk = nc.dram_tensor("xk", [D, S], F16, kind="ExternalInput")
    xv = nc.dram_tensor("xv", [D, S], F16, kind="ExternalInput")
    wq = nc.dram_tensor("wq", [D, E], F16, kind="ExternalInput")   # Wq[hsl,:].T
    wk = nc.dram_tensor("wk", [D, E], F16, kind="ExternalInput")
    wv = nc.dram_tensor("wv", [D, E], F16, kind="ExternalInput")
    wo = nc.dram_tensor("wo", [E, D], F16, kind="ExternalInput")   # Wo[:,hsl].T
    bq = nc.dram_tensor("bq", [1, E], F16, kind="ExternalInput")
    bk = nc.dram_tensor("bk", [1, E], F16, kind="ExternalInput")
    bv = nc.dram_tensor("bv", [1, E], F16, kind="ExternalInput")
    bo = nc.dram_tensor("bo", [1, D], F16, kind="ExternalInput")
    onesr = nc.dram_tensor("onesr", [1, 512], F16, kind="ExternalInput")
    onescol = nc.dram_tensor("onescol", [128, HPC], F16, kind="ExternalInput")
    ident = nc.dram_tensor("ident", [128, 128], F16, kind="ExternalInput")
    # mask windows, transposed ([k, q]); causal: per-kt 512-wide window
    if causal:
        maskw = nc.dram_tensor("maskw", [NKT, 128, 512], F16, kind="ExternalInput")
    else:
        maskw = nc.dram_tensor("maskw", [NKT, 128, S], F16, kind="ExternalInput")

    attnT = nc.dram_tensor("attnT", [HPC, S, S], F32, kind="ExternalOutput")
    outT = nc.dram_tensor("outT", [D, S], F32, kind="ExternalOutput")

    with tile.TileContext(nc) as tc:
        with tc.tile_pool(name="const", bufs=1) as constp:
            wo_t = [constp.tile([128, D], F16, tag=f"wo{i}", name=f"wo{i}")
                    for i in range(2)]
            for i in range(2):
                nc.gpsimd.dma_start(wo_t[i][:], wo[i * 128:(i + 1) * 128, :])
            bo_t = constp.tile([1, D], F16, tag="bo", name="bo")
            nc.gpsimd.dma_start(bo_t[:], bo[:])
            ones_t = constp.tile([1, 512], F16, tag="ones", name="ones")
            nc.gpsimd.dma_start(ones_t[:], onesr[:])
            id_t = constp.tile([128, 128], F16, tag="ident", name="ident")
            nc.gpsimd.dma_start(id_t[:], ident[:])

            qhT = [constp.tile([128, S], F16, tag=f"qhT{i}", name=f"qhT{i}")
                   for i in range(2)]
            khT = [constp.tile([128, S], F16, tag=f"khT{i}", name=f"khT{i}")
                   for i in range(2)]
            vha = [constp.tile([128, HPC * (DK + 1)], F16, tag=f"vha{k}",
                               name=f"vha{k}") for k in range(NKT)]
            ctx = [constp.tile([128, S], F16, tag=f"ctx{i}", name=f"ctx{i}")
                   for i in range(2)]

            # ---------------- phase 1: projections ----------------
            with (
                tc.tile_pool(name="wproj", bufs=1) as wprojp,
                tc.tile_pool(name="xs", bufs=6) as xsp,
                tc.tile_pool(name="pp", bufs=4, space="PSUM") as pp,
            ):
                wq_t = wprojp.tile([128, ND * E], F16, tag="wq", name="wq")
                wk_t = wprojp.tile([128, ND * E], F16, tag="wk", name="wk")
                wv_t = wprojp.tile([128, ND * E], F16, tag="wv", name="wv")
                for w_t, w in ((wq_t, wq), (wk_t, wk), (wv_t, wv)):
                    nc.gpsimd.dma_start(
                        w_t[:].rearrange("p (t e) -> p t e", e=E),
                        w[:].rearrange("(t p) e -> p t e", t=ND))
                bq_t = wprojp.tile([1, E], F16, tag="bq", name="bq")
                bk_t = wprojp.tile([1, E], F16, tag="bk", name="bk")
                bv_t = wprojp.tile([1, E], F16, tag="bv", name="bv")
                nc.gpsimd.dma_start(bq_t[:], bq[:])
                nc.gpsimd.dma_start(bk_t[:], bk[:])
                nc.gpsimd.dma_start(bv_t[:], bv[:])

                # qh^T / kh^T: out[e_tile(128), s]; lhsT = w[d, e] slice
                for src, w_t, b_t, dst in ((xq, wq_t, bq_t, qhT),
                                           (xk, wk_t, bk_t, khT)):
                    for sb in range(NSB):
                        xt = []
                        for d4 in range(2):  # 4 d-slices per tile
                            t = xsp.tile([128, 2048], F16, tag="xs", name="xs")
                            nc.sync.dma_start(
                                t[:].rearrange("p (t q) -> p t q", q=512),
                                src[d4 * 512:(d4 + 1) * 512,
                                    sb * 512:(sb + 1) * 512]
                                .rearrange("(t p) q -> p t q", t=4))
                            xt.append(t)
                        for hp in range(2):
                            ps = pp.tile([128, 512], F32, tag="pp", name="pp")
                            nc.tensor.matmul(
                                ps[:], b_t[0:1, hp * 128:(hp + 1) * 128],
                                ones_t[:], start=True, stop=False)
                            for ds in range(ND):
                                nc.tensor.matmul(
                                    ps[:],
                                    w_t[:, ds * E + hp * 128:
                                        ds * E + (hp + 1) * 128],
                                    xt[ds // 4][:, (ds % 4) * 512:
                                                (ds % 4 + 1) * 512],
                                    start=False, stop=(ds == ND - 1))
                            nc.scalar.copy(dst[hp][:, sb * 512:(sb + 1) * 512],
                                           ps[:])
                # vh: out[s_tile(128), e]; lhsT = x^T[d, s_tile], rhs = wv[d, e]
                for sb in range(NSB):
                    xt = []
                    for d4 in range(2):
                        t = xsp.tile([128, 2048], F16, tag="xs", name="xs")
                        nc.sync.dma_start(
                            t[:].rearrange("p (t q) -> p t q", q=512),
                            xv[d4 * 512:(d4 + 1) * 512,
                               sb * 512:(sb + 1) * 512]
                            .rearrange("(t p) q -> p t q", t=4))
                        xt.append(t)
                    for st in range(4):
                        kt = sb * 4 + st
                        ps = pp.tile([128, E], F32, tag="pp", name="pp")
                        nc.tensor.matmul(ps[:], ones_t[0:1, 0:128], bv_t[:],
                                         start=True, stop=False)
                        for ds in range(ND):
                            nc.tensor.matmul(
                                ps[:],
                                xt[ds // 4][:, (ds % 4) * 512 + st * 128:
                                            (ds % 4) * 512 + (st + 1) * 128],
                                wv_t[:, ds * E:(ds + 1) * E],
                                start=False, stop=(ds == ND - 1))
                        dstv = vha[kt][:].rearrange("p (h c) -> p h c", c=DK + 1)
                        nc.scalar.copy(
                            dstv[:, :, 0:DK],
                            ps[:].rearrange("p (h c) -> p h c", c=DK))
                        nc.gpsimd.dma_start(
                            dstv[:, :, DK:DK + 1],
                            onescol[:].rearrange("p (h o) -> p h o", o=1))

            # ------------- phase 2+3: attention + output proj -------------
            with (
                tc.tile_pool(name="pt", bufs=18) as ptp,
                tc.tile_pool(name="stage", bufs=5) as stagep,
                tc.tile_pool(name="rbp", bufs=4) as rbp,
                tc.tile_pool(name="rrow", bufs=4) as rrowp,
                tc.tile_pool(name="mw", bufs=6) as mwp,
                tc.tile_pool(name="ostage", bufs=3) as ostagep,
                tc.tile_pool(name="pspair", bufs=2, space="PSUM") as pspair,
                tc.tile_pool(name="psctx", bufs=2, space="PSUM") as psctx,
                tc.tile_pool(name="po", bufs=2, space="PSUM") as pop,
            ):
                for qb in range(NSB):
                    nkt = 4 * qb + 4 if causal else NKT
                    masked = set(range(4 * qb, 4 * qb + 4)) if causal \
                        else set(range(NKT))
                    mt = {}
                    for kt in sorted(masked):
                        m = mwp.tile([128, 512], F16, tag="mw", name="mw")
                        if causal:
                            nc.gpsimd.dma_start(m[:], maskw[kt, :, :])
                        else:
                            nc.gpsimd.dma_start(
                                m[:], maskw[kt, :, qb * 512:(qb + 1) * 512])
                        mt[kt] = m

                    for hp in range(2):
                        psc = [psctx.tile([65, 512], F32, tag="psctx",
                                          name="psctx") for _ in range(2)]
                        head_pts = {0: [], 1: []}
                        for kt0 in range(0, nkt, 2):
                            kt1 = kt0 + 1
                            for h01 in range(2):
                                lh = 2 * hp + h01
                                pss = pspair.tile([128, 1024], F32,
                                                  tag="pspair", name="pspair")
                                for seg, kt in ((0, kt0), (1, kt1)):
                                    sl = pss[:, seg * 512:(seg + 1) * 512]
                                    nc.tensor.matmul(
                                        sl,
                                        khT[hp][64 * h01:64 * h01 + 64,
                                                kt * 128:(kt + 1) * 128],
                                        qhT[hp][64 * h01:64 * h01 + 64,
                                                qb * 512:(qb + 1) * 512],
                                        start=True, stop=(kt not in masked),
                                        tile_position=(64 * h01, 0))
                                    if kt in masked:
                                        nc.tensor.matmul(
                                            sl, id_t[:], mt[kt][:],
                                            start=False, stop=True)
                                ptt = ptp.tile([128, 1024], F16, tag="pt",
                                               name="pt")
                                nc.scalar.activation(ptt[:], pss[:], Exp,
                                                     scale=0.125)
                                for seg, kt in ((0, kt0), (1, kt1)):
                                    nc.tensor.matmul(
                                        psc[h01][0:65, :],
                                        vha[kt][:, 65 * lh:65 * lh + 65],
                                        ptt[:, seg * 512:(seg + 1) * 512],
                                        start=(kt == 0), stop=(kt == nkt - 1))
                                head_pts[h01].append((ptt, kt0))

                        for h01 in range(2):
                            lh = 2 * hp + h01
                            rrow = rrowp.tile([1, 512], F32, tag="rrow",
                                              name="rrow")
                            nc.vector.reciprocal(rrow[0:1, :], psc[h01][64:65, :])
                            rbt = rbp.tile([128, 1024], F32, tag="rb", name="rb")
                            nc.gpsimd.partition_broadcast(rbt[:, 0:512],
                                                          rrow[0:1, :])
                            nc.gpsimd.partition_broadcast(rbt[:, 512:1024],
                                                          rrow[0:1, :])
                            nc.vector.tensor_mul(
                                ctx[hp][64 * h01:64 * h01 + 64,
                                        qb * 512:(qb + 1) * 512],
                                psc[h01][0:64, :], rbt[0:64, 0:512])
                            for i, (ptt, kt0) in enumerate(head_pts[h01]):
                                st = stagep.tile([128, 1024], F32, tag="stage",
                                                 name="stage")
                                eng = nc.gpsimd if i % 3 == 2 else nc.vector
                                eng.tensor_mul(st[:], ptt[:], rbt[:])
                                nc.sync.dma_start(
                                    attnT[lh, kt0 * 128:(kt0 + 2) * 128,
                                          qb * 512:(qb + 1) * 512]
                                    .rearrange("(t p) q -> p t q", t=2),
                                    st[:].rearrange("p (t q) -> p t q", q=512))

                    # output projection for this qb
                    for m in range(D // 128):
                        ps = pop.tile([128, 512], F32, tag="po", name="po")
                        nc.tensor.matmul(ps[:], bo_t[0:1, m * 128:(m + 1) * 128],
                                         ones_t[:], start=True, stop=False)
                        for i in range(2):
                            nc.tensor.matmul(
                                ps[:], wo_t[i][:, m * 128:(m + 1) * 128],
                                ctx[i][:, qb * 512:(qb + 1) * 512],
                                start=False, stop=(i == 1))
                        ost = ostagep.tile([128, 512], F32, tag="ostage",
                                           name="ostage")
                        nc.scalar.copy(ost[:], ps[:])
                        nc.sync.dma_start(outT[m * 128:(m + 1) * 128,
                                               qb * 512:(qb + 1) * 512], ost[:])

    nc.compile()
    return nc


def _get_nc(causal: bool):
    if causal not in _cache:
        _cache[causal] = _build(causal)
    return _cache[causal]


def _prep_inputs(q, k, v, attn_mask, Wq, bq, Wk, bk, Wv, bv, Wo, bo):
    m2 = np.asarray(attn_mask).reshape(S, S)
    causal = bool(np.array_equal((m2 != 0), np.tril(np.ones((S, S), bool))))

    if causal:
        maskw = np.zeros((NKT, 128, 512), np.float32)
        for kt in range(NKT):
            r = kt % 4
            maskw[kt, :, 0:128 * r] = NEG
            sub = m2[kt * 128:(kt + 1) * 128, kt * 128:(kt + 1) * 128]  # [q,k]
            maskw[kt, :, 128 * r:128 * (r + 1)] = \
                np.where(sub == 0, np.float32(NEG), np.float32(0.0)).T
    else:
        maskw = np.empty((NKT, 128, S), np.float32)
        for kt in range(NKT):
            sub = m2[:, kt * 128:(kt + 1) * 128]  # [q, k]
            maskw[kt] = np.where(sub == 0, np.float32(NEG), np.float32(0.0)).T
    maskw = maskw.astype(np.float16)

    xT = {}
    for name, x in (("xq", q), ("xk", k), ("xv", v)):
        xT[name] = [np.ascontiguousarray(np.asarray(x)[b].T).astype(np.float16)
                    for b in range(B)]

    onesr16 = np.ones((1, 512), np.float16)
    onescol16 = np.ones((128, HPC), np.float16)
    ident16 = np.eye(128, dtype=np.float16)

    Wq, Wk, Wv, Wo = (np.asarray(a, np.float32) for a in (Wq, Wk, Wv, Wo))
    bqv, bkv, bvv, bov = (np.asarray(a, np.float32) for a in (bq, bk, bv, bo))

    in_maps = []
    for c in range(NCORES):
        b = c // 4
        hs = slice(E * (c % 4), E * (c % 4 + 1))
        im = {
            "xq": xT["xq"][b], "xk": xT["xk"][b], "xv": xT["xv"][b],
            "wq": np.ascontiguousarray(Wq[hs, :].T).astype(np.float16),
            "wk": np.ascontiguousarray(Wk[hs, :].T).astype(np.float16),
            "wv": np.ascontiguousarray(Wv[hs, :].T).astype(np.float16),
            "wo": np.ascontiguousarray(Wo[:, hs].T).astype(np.float16),
            "bq": bqv[hs][None, :].astype(np.float16),
            "bk": bkv[hs][None, :].astype(np.float16),
            "bv": bvv[hs][None, :].astype(np.float16),
            "bo": (bov[None, :].astype(np.float16) if c % 4 == 0
                   else np.zeros((1, D), np.float16)),
            "onesr": onesr16, "onescol": onescol16, "ident": ident16,
            "maskw": maskw,
        }
        in_maps.append(im)
    return causal, in_maps


def _gather(results):
    attn = np.empty((B, H, S, S), np.float32)
    output = np.empty((B, S, D), np.float32)
    for b in range(B):
        acc = None
        for g in range(4):
            c = b * 4 + g
            r = results[c]
            at = r["attnT"]
            for i in range(HPC):
                attn[b, HPC * (c % 4) + i] = at[i].T
            acc = r["outT"] if acc is None else acc + r["outT"]
        output[b] = acc.T
    return output, attn


def run(trace=False, **inputs):
    causal, in_maps = _prep_inputs(**inputs)
    nc = _get_nc(causal)
    res = bass_utils.run_bass_kernel_spmd(
        nc, in_maps, core_ids=list(range(NCORES)), trace=trace)
    output, attn = _gather(res.results)
    return (output, attn), res


def kernel(**inputs):
    (output, attn), _ = run(trace=False, **inputs)
    return output, attn


# revision 9
# speedup vs baseline: 1.2168x; 1.0895x over previous
"""Self-contained Trainium2 Bass kernel: causal multi-head attention.

Problem: B=2, S=2048, D=1024, H=16 (DK=64), f32, returns (output, attn).

Sharding over 8 NeuronCores: core c handles batch b = c//4 and the 4 heads
4*(c%4) .. 4*(c%4)+4 (data parallel on B, tensor parallel on heads).  Each
core computes its heads' QKV projections, causal attention (writing its slice
of the attention-probability tensor), and a partial output projection; the
host sums the 4 partial output projections per batch (TP unshard).

On-device layout is "transposed space": activations are [channel, seq], so
score tiles come out as s^T[k, q] and feed the P@V matmul with no on-chip
transposes; the host pre-transposes inputs and de-transposes outputs.

Compute dtype is fp16 (operands; all accumulation f32 in PSUM) — full PE rate
with hideable weight loads.  The causal mask is applied on the PE itself via
an identity-matmul accumulation (psum += I.T @ maskT, mask bias -60000 so exp
underflows to exactly 0).  Softmax denominators come free as a ones-column
appended to V; normalization is reciprocal + gpsimd partition_broadcast + one
multiply that also produces the f32 attention tile.  Only lower-triangle
[k,q] block-rows are computed; the rest relies on pre-zeroed output buffers.
A non-causal mask falls back to a general variant computing every block.
"""

import sys
import types

if "/opt/trn_rl_repo" not in sys.path:
    sys.path.insert(0, "/opt/trn_rl_repo")

import numpy as np


def _install_ntff_hook():
    """Recreate antenv.axon_hooks (missing in this image) so that
    run_bass_kernel_spmd(trace=True) can capture NTFF profiles."""
    if "antenv.axon_hooks" in sys.modules:
        return
    try:
        from trn_agent_boot.trn_boot import _ntff_profile_via_ctypes
    except ImportError:
        return
    try:
        hook = _ntff_profile_via_ctypes("/opt/axon/libaxon_pjrt.so")
    except OSError:
        hook = None
    mod = types.ModuleType("antenv.axon_hooks")
    mod.get_axon_ntff_profile_hook = lambda: hook
    mod.set_axon_ntff_profile_hook = lambda h: None
    sys.modules["antenv.axon_hooks"] = mod


_install_ntff_hook()

import concourse.bacc as bacc
import concourse.mybir as mybir
import concourse.tile as tile
from concourse import bass_utils

B, S, D, H = 2, 2048, 1024, 16
DK = D // H          # 64
NCORES = 8
HPC = 4              # heads per core
E = HPC * DK         # 256 proj channels per core
NEG = -60000.0       # fp16-representable; exp(0.125*(s+NEG)) == 0 in f32
NKT = S // 128       # 16 key tiles
NSB = S // 512       # 4 seq blocks
ND = D // 128        # 8 contraction slices

F32 = mybir.dt.float32
F16 = mybir.dt.float16
Exp = mybir.ActivationFunctionType.Exp

_cache = {}


def _build(causal: bool):
    nc = bacc.Bacc("TRN2", target_bir_lowering=False, debug=False,
                   num_devices=NCORES)

    # ---- I/O (inputs fp16; outputs f32) ----
    xq = nc.dram_tensor("xq", [D, S], F16, kind="ExternalInput")   # q[b].T
    xk = nc.dram_tensor("xk", [D, S], F16, kind="ExternalInput")
    xv = nc.dram_tensor("xv", [D, S], F16, kind="ExternalInput")
    wq = nc.dram_tensor("wq", [D, E], F16, kind="ExternalInput")   # Wq[hsl,:].T
    wk = nc.dram_tensor("wk", [D, E], F16, kind="ExternalInput")
    wv = nc.dram_tensor("wv", [D, E], F16, kind="ExternalInput")
    wo = nc.dram_tensor("wo", [E, D], F16, kind="ExternalInput")   # Wo[:,hsl].T
    bq = nc.dram_tensor("bq", [1, E], F16, kind="ExternalInput")
    bk = nc.dram_tensor("bk", [1, E], F16, kind="ExternalInput")
    bv = nc.dram_tensor("bv", [1, E], F16, kind="ExternalInput")
    bo = nc.dram_tensor("bo", [1, D], F16, kind="ExternalInput")
    onesr = nc.dram_tensor("onesr", [1, 512], F16, kind="ExternalInput")
    onescol = nc.dram_tensor("onescol", [128, HPC], F16, kind="ExternalInput")
    ident = nc.dram_tensor("ident", [128, 128], F16, kind="ExternalInput")
    # mask windows, transposed ([k, q]); causal: per-kt 512-wide window
    if causal:
        maskw = nc.dram_tensor("maskw", [NKT, 128, 512], F16, kind="ExternalInput")
    else:
        maskw = nc.dram_tensor("maskw", [NKT, 128, S], F16, kind="ExternalInput")

    attnT = nc.dram_tensor("attnT", [S, HPC, S], F32, kind="ExternalOutput")
    outT = nc.dram_tensor("outT", [D, S], F32, kind="ExternalOutput")

    with tile.TileContext(nc) as tc:
        with tc.tile_pool(name="const", bufs=1) as constp:
            wo_t = [constp.tile([128, D], F16, tag=f"wo{i}", name=f"wo{i}")
                    for i in range(2)]
            for i in range(2):
                nc.gpsimd.dma_start(wo_t[i][:], wo[i * 128:(i + 1) * 128, :])
            bo_t = constp.tile([1, D], F16, tag="bo", name="bo")
            nc.gpsimd.dma_start(bo_t[:], bo[:])
            ones_t = constp.tile([1, 512], F16, tag="ones", name="ones")
            nc.gpsimd.dma_start(ones_t[:], onesr[:])
            id_t = constp.tile([128, 128], F16, tag="ident", name="ident")
            nc.gpsimd.dma_start(id_t[:], ident[:])

            qhT = [constp.tile([128, S], F16, tag=f"qhT{i}", name=f"qhT{i}")
                   for i in range(2)]
            khT = [constp.tile([128, S], F16, tag=f"khT{i}", name=f"khT{i}")
                   for i in range(2)]
            vha = [constp.tile([128, HPC * (DK + 1)], F16, tag=f"vha{k}",
                               name=f"vha{k}") for k in range(NKT)]
            ctx = [constp.tile([128, S], F16, tag=f"ctx{i}", name=f"ctx{i}")
                   for i in range(2)]

            # ---------------- phase 1: projections ----------------
            with (
                tc.tile_pool(name="wproj", bufs=1) as wprojp,
                tc.tile_pool(name="xs", bufs=6) as xsp,
                tc.tile_pool(name="pp", bufs=4, space="PSUM") as pp,
            ):
                wq_t = wprojp.tile([128, ND * E], F16, tag="wq", name="wq")
                wk_t = wprojp.tile([128, ND * E], F16, tag="wk", name="wk")
                wv_t = wprojp.tile([128, ND * E], F16, tag="wv", name="wv")
                for w_t, w in ((wq_t, wq), (wk_t, wk), (wv_t, wv)):
                    nc.gpsimd.dma_start(
                        w_t[:].rearrange("p (t e) -> p t e", e=E),
                        w[:].rearrange("(t p) e -> p t e", t=ND))
                bq_t = wprojp.tile([1, E], F16, tag="bq", name="bq")
                bk_t = wprojp.tile([1, E], F16, tag="bk", name="bk")
                bv_t = wprojp.tile([1, E], F16, tag="bv", name="bv")
                nc.gpsimd.dma_start(bq_t[:], bq[:])
                nc.gpsimd.dma_start(bk_t[:], bk[:])
                nc.gpsimd.dma_start(bv_t[:], bv[:])

                # vh: out[s_tile(128), e]; lhsT = x^T[d, s_tile], rhs = wv[d, e]
                for sb in range(NSB):
                    xt = []
                    for d4 in range(2):
                        t = xsp.tile([128, 2048], F16, tag="xs", name="xs")
                        nc.sync.dma_start(
                            t[:].rearrange("p (t q) -> p t q", q=512),
                            xv[d4 * 512:(d4 + 1) * 512,
                               sb * 512:(sb + 1) * 512]
                            .rearrange("(t p) q -> p t q", t=4))
                        xt.append(t)
                    for st in range(4):
                        kt = sb * 4 + st
                        ps = pp.tile([128, E], F32, tag="pp", name="pp")
                        nc.tensor.matmul(ps[:], ones_t[0:1, 0:128], bv_t[:],
                                         start=True, stop=False)
                        for ds in range(ND):
                            nc.tensor.matmul(
                                ps[:],
                                xt[ds // 4][:, (ds % 4) * 512 + st * 128:
                                            (ds % 4) * 512 + (st + 1) * 128],
                                wv_t[:, ds * E:(ds + 1) * E],
                                start=False, stop=(ds == ND - 1))
                        dstv = vha[kt][:].rearrange("p (h c) -> p h c", c=DK + 1)
                        nc.scalar.copy(
                            dstv[:, :, 0:DK],
                            ps[:].rearrange("p (h c) -> p h c", c=DK))
                        nc.gpsimd.dma_start(
                            dstv[:, :, DK:DK + 1],
                            onescol[:].rearrange("p (h o) -> p h o", o=1))

                # kh^T then qh^T: out[e_tile(128), s]; lhsT = w[d, e] slice
                for src, w_t, b_t, dst in ((xk, wk_t, bk_t, khT),
                                           (xq, wq_t, bq_t, qhT)):
                    for sb in range(NSB):
                        xt = []
                        for d4 in range(2):  # 4 d-slices per tile
                            t = xsp.tile([128, 2048], F16, tag="xs", name="xs")
                            nc.sync.dma_start(
                                t[:].rearrange("p (t q) -> p t q", q=512),
                                src[d4 * 512:(d4 + 1) * 512,
                                    sb * 512:(sb + 1) * 512]
                                .rearrange("(t p) q -> p t q", t=4))
                            xt.append(t)
                        for hp in range(2):
                            ps = pp.tile([128, 512], F32, tag="pp", name="pp")
                            nc.tensor.matmul(
                                ps[:], b_t[0:1, hp * 128:(hp + 1) * 128],
                                ones_t[:], start=True, stop=False)
                            for ds in range(ND):
                                nc.tensor.matmul(
                                    ps[:],
                                    w_t[:, ds * E + hp * 128:
                                        ds * E + (hp + 1) * 128],
                                    xt[ds // 4][:, (ds % 4) * 512:
                                                (ds % 4 + 1) * 512],
                                    start=False, stop=(ds == ND - 1))
                            nc.scalar.copy(dst[hp][:, sb * 512:(sb + 1) * 512],
                                           ps[:])
            # ------------- phase 2+3: attention + output proj -------------
            with (
                tc.tile_pool(name="pt", bufs=18) as ptp,
                tc.tile_pool(name="stage", bufs=5) as stagep,
                tc.tile_pool(name="rbp", bufs=4) as rbp,
                tc.tile_pool(name="rrow", bufs=4) as rrowp,
                tc.tile_pool(name="mw", bufs=6) as mwp,
                tc.tile_pool(name="dscr", bufs=4, space="DRAM") as dscrp,
                tc.tile_pool(name="ostage", bufs=3) as ostagep,
                tc.tile_pool(name="pspair", bufs=2, space="PSUM") as pspair,
                tc.tile_pool(name="psctx", bufs=2, space="PSUM") as psctx,
                tc.tile_pool(name="po", bufs=2, space="PSUM") as pop,
            ):
                for qb in range(NSB):
                    nkt = 4 * qb + 4 if causal else NKT
                    masked = set(range(4 * qb, 4 * qb + 4)) if causal \
                        else set(range(NKT))
                    mt = {}
                    for kt in sorted(masked):
                        m = mwp.tile([128, 512], F16, tag="mw", name="mw")
                        if causal:
                            nc.gpsimd.dma_start(m[:], maskw[kt, :, :])
                        else:
                            nc.gpsimd.dma_start(
                                m[:], maskw[kt, :, qb * 512:(qb + 1) * 512])
                        mt[kt] = m

                    for hp in range(2):
                        psc = [psctx.tile([65, 512], F32, tag="psctx",
                                          name="psctx") for _ in range(2)]
                        kt_pts = []
                        for kt in range(nkt):
                            pss = pspair.tile([128, 1024], F32,
                                              tag="pspair", name="pspair")
                            # both heads' scores, adjacent + disjoint row groups
                            for h01 in range(2):
                                nc.tensor.matmul(
                                    pss[:, h01 * 512:(h01 + 1) * 512],
                                    khT[hp][64 * h01:64 * h01 + 64,
                                            kt * 128:(kt + 1) * 128],
                                    qhT[hp][64 * h01:64 * h01 + 64,
                                            qb * 512:(qb + 1) * 512],
                                    start=True, stop=(kt not in masked),
                                    tile_position=(64 * h01, 0))
                            if kt in masked:
                                for h01 in range(2):
                                    nc.tensor.matmul(
                                        pss[:, h01 * 512:(h01 + 1) * 512],
                                        id_t[:], mt[kt][:],
                                        start=False, stop=True)
                            ptt = ptp.tile([128, 1024], F16, tag="pt",
                                           name="pt")
                            nc.scalar.activation(ptt[:], pss[:], Exp,
                                                 scale=0.125)
                            for h01 in range(2):
                                lh = 2 * hp + h01
                                nc.tensor.matmul(
                                    psc[h01][0:65, :],
                                    vha[kt][:, 65 * lh:65 * lh + 65],
                                    ptt[:, h01 * 512:(h01 + 1) * 512],
                                    start=(kt == 0), stop=(kt == nkt - 1))
                            kt_pts.append((ptt, kt))

                        rbt = rbp.tile([128, 1024], F32, tag="rb", name="rb")
                        for h01 in range(2):
                            rrow = rrowp.tile([1, 512], F32, tag="rrow",
                                              name="rrow")
                            nc.vector.reciprocal(rrow[0:1, :], psc[h01][64:65, :])
                            dscr = dscrp.tile([1, 512], F32, tag="dscr",
                                              name="dscr")
                            nc.gpsimd.dma_start(dscr[:], rrow[0:1, :])
                            nc.scalar.dma_start(
                                rbt[:, h01 * 512:(h01 + 1) * 512],
                                dscr[:].to_broadcast((128, 512)))
                            nc.vector.tensor_mul(
                                ctx[hp][64 * h01:64 * h01 + 64,
                                        qb * 512:(qb + 1) * 512],
                                psc[h01][0:64, :],
                                rbt[0:64, h01 * 512:h01 * 512 + 512])
                        for i, (ptt, kt) in enumerate(kt_pts):
                            st = stagep.tile([128, 1024], F32, tag="stage",
                                             name="stage")
                            eng = nc.gpsimd if i % 3 == 2 else nc.vector
                            eng.tensor_mul(st[:], ptt[:], rbt[:])
                            nc.sync.dma_start(
                                attnT[kt * 128:(kt + 1) * 128,
                                      2 * hp:2 * hp + 2,
                                      qb * 512:(qb + 1) * 512],
                                st[:].rearrange("p (t q) -> p t q", q=512))

                    # output projection for this qb
                    for m in range(D // 128):
                        ps = pop.tile([128, 512], F32, tag="po", name="po")
                        nc.tensor.matmul(ps[:], bo_t[0:1, m * 128:(m + 1) * 128],
                                         ones_t[:], start=True, stop=False)
                        for i in range(2):
                            nc.tensor.matmul(
                                ps[:], wo_t[i][:, m * 128:(m + 1) * 128],
                                ctx[i][:, qb * 512:(qb + 1) * 512],
                                start=False, stop=(i == 1))
                        ost = ostagep.tile([128, 512], F32, tag="ostage",
                                           name="ostage")
                        nc.scalar.copy(ost[:], ps[:])
                        nc.sync.dma_start(outT[m * 128:(m + 1) * 128,
                                               qb * 512:(qb + 1) * 512], ost[:])

    nc.compile()
    return nc


def _get_nc(causal: bool):
    if causal not in _cache:
        _cache[causal] = _build(causal)
    return _cache[causal]


def _prep_inputs(q, k, v, attn_mask, Wq, bq, Wk, bk, Wv, bv, Wo, bo):
    m2 = np.asarray(attn_mask).reshape(S, S)
    causal = bool(np.array_equal((m2 != 0), np.tril(np.ones((S, S), bool))))

    if causal:
        maskw = np.zeros((NKT, 128, 512), np.float32)
        for kt in range(NKT):
            r = kt % 4
            maskw[kt, :, 0:128 * r] = NEG
            sub = m2[kt * 128:(kt + 1) * 128, kt * 128:(kt + 1) * 128]  # [q,k]
            maskw[kt, :, 128 * r:128 * (r + 1)] = \
                np.where(sub == 0, np.float32(NEG), np.float32(0.0)).T
    else:
        maskw = np.empty((NKT, 128, S), np.float32)
        for kt in range(NKT):
            sub = m2[:, kt * 128:(kt + 1) * 128]  # [q, k]
            maskw[kt] = np.where(sub == 0, np.float32(NEG), np.float32(0.0)).T
    maskw = maskw.astype(np.float16)

    xT = {}
    for name, x in (("xq", q), ("xk", k), ("xv", v)):
        xT[name] = [np.ascontiguousarray(np.asarray(x)[b].T).astype(np.float16)
                    for b in range(B)]

    onesr16 = np.ones((1, 512), np.float16)
    onescol16 = np.ones((128, HPC), np.float16)
    ident16 = np.eye(128, dtype=np.float16)

    Wq, Wk, Wv, Wo = (np.asarray(a, np.float32) for a in (Wq, Wk, Wv, Wo))
    bqv, bkv, bvv, bov = (np.asarray(a, np.float32) for a in (bq, bk, bv, bo))

    in_maps = []
    for c in range(NCORES):
        b = c // 4
        hs = slice(E * (c % 4), E * (c % 4 + 1))
        im = {
            "xq": xT["xq"][b], "xk": xT["xk"][b], "xv": xT["xv"][b],
            "wq": np.ascontiguousarray(Wq[hs, :].T).astype(np.float16),
            "wk": np.ascontiguousarray(Wk[hs, :].T).astype(np.float16),
            "wv": np.ascontiguousarray(Wv[hs, :].T).astype(np.float16),
            "wo": np.ascontiguousarray(Wo[:, hs].T).astype(np.float16),
            "bq": bqv[hs][None, :].astype(np.float16),
            "bk": bkv[hs][None, :].astype(np.float16),
            "bv": bvv[hs][None, :].astype(np.float16),
            "bo": (bov[None, :].astype(np.float16) if c % 4 == 0
                   else np.zeros((1, D), np.float16)),
            "onesr": onesr16, "onescol": onescol16, "ident": ident16,
            "maskw": maskw,
        }
        in_maps.append(im)
    return causal, in_maps


def _gather(results):
    attn = np.empty((B, H, S, S), np.float32)
    output = np.empty((B, S, D), np.float32)
    for b in range(B):
        acc = None
        for g in range(4):
            c = b * 4 + g
            r = results[c]
            at = r["attnT"]
            for i in range(HPC):
                attn[b, HPC * (c % 4) + i] = at[:, i, :].T
            acc = r["outT"] if acc is None else acc + r["outT"]
        output[b] = acc.T
    return output, attn


def run(trace=False, **inputs):
    causal, in_maps = _prep_inputs(**inputs)
    nc = _get_nc(causal)
    res = bass_utils.run_bass_kernel_spmd(
        nc, in_maps, core_ids=list(range(NCORES)), trace=trace)
    output, attn = _gather(res.results)
    return (output, attn), res


def kernel(**inputs):
    (output, attn), _ = run(trace=False, **inputs)
    return output, attn
